# revision 37
# baseline (speedup 1.0000x reference)
"""Trainium2 Bass kernel: 3x3 VALID conv2d, stride 1.

Full input [32, 64, 112, 112] f32 + weights [128, 64, 3, 3] f32
-> output [32, 128, 110, 110] f32.

Data-parallel across 8 NeuronCores: 4 images per core.

Per-core formulation: conv as PE matmuls, out = lhsT.T @ rhs with
M (out partitions) = 128 output channels,
N (moving free dim) = 4 input-width rows = 448 (<= 512, one PSUM bank).
The 2 rightmost columns of each 112-wide row are conv garbage; the
PSUM->SBUF copy compacts to the valid 110 columns.

Each image lives in SBUF twice (host-duplicated layout): copy A = rows
0..111 in partitions 0..63, copy B = rows 1..111 in partitions 64..127.
Taps (ky=0,kx)+(ky=1,kx) are applied by one K=128 matmul per kx (3
matmuls).  The ky=2 taps are K=64 matmuls: for even chunks they read
copy A at +2 rows (PE row-group 0-1, tile_position (0,0)), for odd
chunks copy B at +1 row (row-group 2-3, tile_position (64,0)).  The
hardware runs a (0,0)/(64,0) pair CONCURRENTLY (independent 32x32
sub-arrays), so the pair costs one matmul slot: 4.5 slots per chunk
instead of 6.

Inputs are cast to fp16 on the host (fp16 streams the PE at full rate,
fp32 PSUM accumulation, rel err ~4e-4).  The output is written as fp16
(halves output HBM traffic + drain tail) and cast back to fp32 on the
host.

Input DMAs ride the HWDGE rings (sync + scalar engines) instead of the
SWDGE/gpsimd path, whose Q7 descriptor generation (~0.7us per DMA)
delayed the first matmul by ~5us.  The first band of image 0 is small
(6 rows) so compute starts as soon as possible.

Schedule: chunks are processed in groups of 8 across the 8 PSUM banks,
weight-plane-major (kx outer), so consecutive matmuls hit different
banks (drain overlaps fill) and reuse the same stationary weights.
"""

import numpy as np

B_FULL = 32
N_CORES = 8
B_CORE = B_FULL // N_CORES  # 4 images per core
C_IN = 64
C_OUT = 128
H = W = 112
OH = OW = 110
XLEN = H * W + 2  # +2 pad so ky=2/kx>0 reads of the last chunk stay in bounds

_NC = None


def _chunks():
    # per image: 27 chunks of 4 output rows + 1 of 2 rows = 110
    rows_list = [4] * 27 + [2]
    out = []
    for b in range(B_CORE):
        y0 = 0
        for r in rows_list:
            out.append((b, y0, r))
            y0 += r
        assert y0 == OH
    return out


def _build():
    from contextlib import ExitStack

    import concourse.tile as tile
    from concourse import bacc, mybir

    nc = bacc.Bacc("TRN2", target_bir_lowering=False, debug=False)
    # host-duplicated layout: [b, s*64+ci, h*112+w] with s=0 -> row h,
    # s=1 -> row h+1 (see kernel()); full 128-partition DMAs use all 16
    # SBUF ports and need no on-device shift copies
    x = nc.dram_tensor(
        "x", [B_CORE, 128, XLEN], mybir.dt.float16, kind="ExternalInput"
    )
    w = nc.dram_tensor("w", [128, 6, 128], mybir.dt.float16, kind="ExternalInput")
    # output rows are stored 112 wide (2 conv-garbage columns kept); the
    # host slices them off.  This makes the PSUM->SBUF copy a flat 1D
    # pattern (the strided 110-column compaction pattern cost ~2x on DVE).
    y = nc.dram_tensor(
        "y", [B_CORE, C_OUT, OH, W], mybir.dt.float16, kind="ExternalOutput"
    )

    chunks = _chunks()
    assert len(chunks) % 8 == 0
    n_groups = len(chunks) // 8

    with tile.TileContext(nc) as tc, ExitStack() as ctx:
        xpool = ctx.enter_context(tc.tile_pool(name="xp", bufs=4))
        wpool = ctx.enter_context(tc.tile_pool(name="wp", bufs=1))
        opool = ctx.enter_context(tc.tile_pool(name="op", bufs=8))
        ppool = ctx.enter_context(tc.tile_pool(name="pp", bufs=8, space="PSUM"))

        wt = wpool.tile([128, 6, 128], mybir.dt.float16)
        nc.sync.dma_start(wt[:], w.ap())

        xa = x.ap()
        ya = y.ap()

        # Input bands: each HWDGE ring is FIFO per engine and each DMA has
        # multi-us push->data->receipt latency, so the bands needed first
        # must be FIRST on their ring.  Image 0 alternates bands across the
        # sync and scalar rings (both rings work on it concurrently);
        # images 1-3 queue strictly behind.  Chunks wait only on the bands
        # they actually read (range-based hazard tracking).
        xtiles = [
            xpool.tile([128, XLEN], mybir.dt.float16, name="xt", tag="xt")
            for _ in range(B_CORE)
        ]

        def load(eng, b, lo, hi):
            end = hi * W if hi < H else XLEN
            eng.dma_start(xtiles[b][:, lo * W : end], xa[b][:, lo * W : end])

        # image 0 alone owns both rings + HBM read bandwidth at the start;
        # images 1-3 are emitted inside the group loop below, so their ring
        # entries sit behind sem-gated group work and only start once the
        # head-critical loads are done.
        load(nc.sync, 0, 0, 6)
        load(nc.scalar, 0, 6, 16)
        load(nc.sync, 0, 16, 30)
        load(nc.scalar, 0, 30, 50)
        load(nc.sync, 0, 50, 80)
        load(nc.scalar, 0, 80, 112)

        # Warm-up: the first real matmul can't start until the runtime
        # preamble + weight/band DMAs complete (~11us).  Run dependency-free
        # dummy matmuls on a zeroed scratch tile in that window so the PE HAM
        # clock-gate reaches 2.4 GHz (and pays its 1.2 GHz cold ramp) before
        # real work arrives.
        scratch = wpool.tile([128, 448], mybir.dt.float16)
        nc.vector.memset(scratch[:], 0.0)
        # shares the 8-bank "pt" rotation; its bank is reused by the 8th real
        # tile of group 0, whose WAR dep lands after the warm-up is long done
        pdummy = ppool.tile([128, 448], mybir.dt.float32, name="pt", tag="pt")
        N_WARM = 17
        for i in range(N_WARM):
            nc.tensor.matmul(
                pdummy[:],
                scratch[:, 0:128],
                scratch[:],
                start=(i == 0),
                stop=(i == N_WARM - 1),
                skip_group_check=True,
            )

        for g in range(n_groups):
            # stagger images 1-3 input loads: emitted after group g-1's
            # sem-gated instructions on each ring, so they queue behind the
            # head-critical image-0 bands instead of competing with them.
            # Images 2-3 go early (g1/g2): by then the head is past, and
            # late arrival of image 2 was the main mid-stream stall on
            # congestion-unlucky cores.
            # both halves ride the scalar ring: after the head it carries
            # no other DMAs, so these sems never queue behind the sem-gated
            # output pushes that congest the sync ring mid-stream (image 1
            # is emitted inside group 0 below, so its push lands in the
            # ring-idle window right after the head bands)
            if 2 <= g <= 3:
                load(nc.scalar, g, 0, 56)
                load(nc.scalar, g, 56, 112)
            gchunks = chunks[g * 8 : (g + 1) * 8]
            pts = [
                ppool.tile([128, 448], mybir.dt.float32, name="pt", tag="pt")
                for _ in range(8)
            ]
            def mm_ky01(j, kx):
                b, y0, rows = gchunks[j]
                n = rows * W
                nc.tensor.matmul(
                    pts[j][:, 0 : n - 2],
                    wt[:, kx, :],
                    xtiles[b][:, y0 * W + kx : y0 * W + kx + n - 2],
                    start=(kx == 0),
                    stop=False,
                    skip_group_check=True,
                )

            def mm_ky2(j, kx):
                # K=64: row-group 0-1 via copy A for even chunks, row-group
                # 2-3 via copy B for odd chunks; an even/odd pair runs
                # concurrently in one matmul slot
                b, y0, rows = gchunks[j]
                n = rows * W
                if j % 2 == 0:
                    lhsT = wt[0:64, 3 + kx, :]
                    rhs = xtiles[b][
                        0:64, (y0 + 2) * W + kx : (y0 + 2) * W + kx + n - 2
                    ]
                else:
                    lhsT = wt[64:128, 3 + kx, :]
                    rhs = xtiles[b][
                        64:128, (y0 + 1) * W + kx : (y0 + 1) * W + kx + n - 2
                    ]
                nc.tensor.matmul(
                    pts[j][:, 0 : n - 2],
                    lhsT,
                    rhs,
                    start=False,
                    stop=(kx == 2),
                    skip_group_check=True,
                )

            def emit_output(lo, hi, eng):
                # copy chunks [lo,hi) of the group into one staging tile and
                # DMA it out as a single contiguous write
                ochunks = gchunks[lo:hi]
                total_rows = sum(r for _, _, r in ochunks)
                ot = opool.tile([128, 32 * W], mybir.dt.float16, tag="ot")
                off = 0
                for jj, (b, y0, rows) in enumerate(ochunks):
                    j = lo + jj
                    n = rows * W
                    if j % 2 == 0:
                        nc.vector.tensor_copy(
                            ot[:, off : off + n - 2], pts[j][:, 0 : n - 2]
                        )
                    else:
                        nc.scalar.copy(ot[:, off : off + n - 2], pts[j][:, 0 : n - 2])
                    off += n
                b0, y00, _ = ochunks[0]
                assert all(b == b0 for b, _, _ in ochunks)
                assert ochunks[-1][1] + ochunks[-1][2] - y00 == total_rows
                eng.dma_start(
                    ya[b0].rearrange("c h w -> c (h w)")[
                        :, y00 * W : y00 * W + total_rows * W
                    ],
                    ot[:, 0 : total_rows * W],
                )

            if g == 0 or g == n_groups - 1:
                # First and last group run chunk-pair-major: slightly worse
                # LDWEIGHTS overlap (weights change every other matmul), but
                # in group 0 it paces input-row consumption to the DMA
                # delivery rate (no stall, no HAM re-throttle), and in the
                # last group it spreads the PSUM stops so the final output
                # drain overlaps the group's own compute.
                for p in range(4):
                    for kx in range(3):
                        mm_ky01(2 * p, kx)
                        mm_ky01(2 * p + 1, kx)
                    for kx in range(3):
                        mm_ky2(2 * p, kx)
                        mm_ky2(2 * p + 1, kx)
                    if g == n_groups - 1:
                        # final drain is latency-critical: emit each pair's
                        # output right after its stop, alternating rings, so
                        # the drain overlaps the group's own compute; the
                        # very last pair splits per-chunk so only the tiny
                        # 2-row chunk's write sits on the critical tail
                        if p == 3:
                            emit_output(6, 7, nc.scalar)
                            emit_output(7, 8, nc.sync)
                        else:
                            emit_output(
                                2 * p, 2 * p + 2, nc.sync if p % 2 else nc.scalar
                            )
                if g == 0:
                    load(nc.scalar, 1, 0, 56)
                    load(nc.scalar, 1, 56, 112)
                    emit_output(0, 4, nc.sync)
                    emit_output(4, 8, nc.sync)
            else:
                # steady state runs kx-major: 8 consecutive matmuls share
                # stationary weights, so every LDWEIGHTS hides under the
                # previous matmul's stream
                for kx in range(3):
                    for j in range(8):
                        mm_ky01(j, kx)
                # ky2 in two half-group sweeps: half 0's PSUM stops land
                # ~1.1us before group end, so its copies drain early and the
                # next group never waits on a bank (2 pair-slots per sub-
                # phase still hide the two 94ns weight loads)
                for h in range(2):
                    for kx in range(3):
                        for j in range(4 * h, 4 * h + 4):
                            mm_ky2(j, kx)
                # batch the whole group's output into one DMA when all 8
                # chunks are in the same image (7KB descriptors instead of
                # 3.5KB halves -> half the per-descriptor overhead); groups
                # that straddle an image boundary split at the half
                if gchunks[0][0] == gchunks[7][0]:
                    emit_output(0, 8, nc.sync)
                else:
                    emit_output(0, 4, nc.sync)
                    emit_output(4, 8, nc.sync)

    nc.compile()
    return nc


def _get_nc():
    global _NC
    if _NC is None:
        _NC = _build()
    return _NC


def _prep_weights(weights: np.ndarray) -> np.ndarray:
    # w6[ci,      kx, co] = w[co, ci, ky=0, kx]   (fused ky=0/1 planes)
    # w6[64+ci,   kx, co] = w[co, ci, ky=1, kx]
    # w6[ci,    3+kx, co] = w[co, ci, ky=2, kx]   (ky=2, A-half row group)
    # w6[64+ci, 3+kx, co] = w[co, ci, ky=2, kx]   (ky=2, B-half row group)
    w = np.asarray(weights, dtype=np.float32)
    wt = w.transpose(1, 2, 3, 0)  # [ci, ky, kx, co]
    w6 = np.zeros((128, 6, 128), np.float32)
    w6[0:64, 0:3, :] = wt[:, 0, :, :]
    w6[64:128, 0:3, :] = wt[:, 1, :, :]
    w6[0:64, 3:6, :] = wt[:, 2, :, :]
    w6[64:128, 3:6, :] = wt[:, 2, :, :]
    return w6.astype(np.float16)


def kernel(input_image: np.ndarray, weights: np.ndarray, _trace: bool = False):
    from concourse.bass_utils import run_bass_kernel_spmd

    nc = _get_nc()
    x16 = np.asarray(input_image).astype(np.float16)  # [32, 64, 112, 112]
    xd = np.zeros((B_FULL, 128, XLEN), np.float16)
    xd[:, :C_IN, : H * W] = x16.reshape(B_FULL, C_IN, H * W)
    xd[:, C_IN:, : (H - 1) * W] = x16[:, :, 1:, :].reshape(B_FULL, C_IN, -1)
    w6 = _prep_weights(weights)
    in_maps = [
        {"x": xd[B_CORE * i : B_CORE * (i + 1)], "w": w6} for i in range(N_CORES)
    ]
    res = run_bass_kernel_spmd(
        nc, in_maps, core_ids=list(range(N_CORES)), trace=_trace
    )
    out = np.concatenate(
        [res.results[i]["y"][:, :, :, :OW] for i in range(N_CORES)], axis=0
    ).astype(np.float32)
    if _trace:
        return out, res
    return out


# revision 38
# speedup vs baseline: 1.0036x; 1.0036x over previous
"""Trainium2 Bass kernel: 3x3 VALID conv2d, stride 1.

Full input [32, 64, 112, 112] f32 + weights [128, 64, 3, 3] f32
-> output [32, 128, 110, 110] f32.

Data-parallel across 8 NeuronCores: 4 images per core.

Per-core formulation: conv as PE matmuls, out = lhsT.T @ rhs with
M (out partitions) = 128 output channels,
N (moving free dim) = 4 input-width rows = 448 (<= 512, one PSUM bank).
The 2 rightmost columns of each 112-wide row are conv garbage; the
PSUM->SBUF copy compacts to the valid 110 columns.

Each image lives in SBUF twice (host-duplicated layout): copy A = rows
0..111 in partitions 0..63, copy B = rows 1..111 in partitions 64..127.
Taps (ky=0,kx)+(ky=1,kx) are applied by one K=128 matmul per kx (3
matmuls).  The ky=2 taps are K=64 matmuls: for even chunks they read
copy A at +2 rows (PE row-group 0-1, tile_position (0,0)), for odd
chunks copy B at +1 row (row-group 2-3, tile_position (64,0)).  The
hardware runs a (0,0)/(64,0) pair CONCURRENTLY (independent 32x32
sub-arrays), so the pair costs one matmul slot: 4.5 slots per chunk
instead of 6.

Inputs are cast to fp16 on the host (fp16 streams the PE at full rate,
fp32 PSUM accumulation, rel err ~4e-4).  The output is written as fp16
(halves output HBM traffic + drain tail) and cast back to fp32 on the
host.

Input DMAs ride the HWDGE rings (sync + scalar engines) instead of the
SWDGE/gpsimd path, whose Q7 descriptor generation (~0.7us per DMA)
delayed the first matmul by ~5us.  The first band of image 0 is small
(6 rows) so compute starts as soon as possible.

Schedule: chunks are processed in groups of 8 across the 8 PSUM banks,
weight-plane-major (kx outer), so consecutive matmuls hit different
banks (drain overlaps fill) and reuse the same stationary weights.
"""

import numpy as np

B_FULL = 32
N_CORES = 8
B_CORE = B_FULL // N_CORES  # 4 images per core
C_IN = 64
C_OUT = 128
H = W = 112
OH = OW = 110
XLEN = H * W + 2  # +2 pad so ky=2/kx>0 reads of the last chunk stay in bounds

_NC = None


def _chunks():
    # per image: 27 chunks of 4 output rows + 1 of 2 rows = 110
    rows_list = [4] * 27 + [2]
    out = []
    for b in range(B_CORE):
        y0 = 0
        for r in rows_list:
            out.append((b, y0, r))
            y0 += r
        assert y0 == OH
    return out


def _build():
    from contextlib import ExitStack

    import concourse.tile as tile
    from concourse import bacc, mybir

    nc = bacc.Bacc("TRN2", target_bir_lowering=False, debug=False)
    # host-duplicated layout: [b, s*64+ci, h*112+w] with s=0 -> row h,
    # s=1 -> row h+1 (see kernel()); full 128-partition DMAs use all 16
    # SBUF ports and need no on-device shift copies
    x = nc.dram_tensor(
        "x", [B_CORE, 128, XLEN], mybir.dt.float16, kind="ExternalInput"
    )
    w = nc.dram_tensor("w", [128, 6, 128], mybir.dt.float16, kind="ExternalInput")
    # output rows are stored 112 wide (2 conv-garbage columns kept); the
    # host slices them off.  This makes the PSUM->SBUF copy a flat 1D
    # pattern (the strided 110-column compaction pattern cost ~2x on DVE).
    y = nc.dram_tensor(
        "y", [B_CORE, C_OUT, OH, W], mybir.dt.float16, kind="ExternalOutput"
    )

    chunks = _chunks()
    assert len(chunks) % 8 == 0
    n_groups = len(chunks) // 8

    with tile.TileContext(nc) as tc, ExitStack() as ctx:
        xpool = ctx.enter_context(tc.tile_pool(name="xp", bufs=4))
        wpool = ctx.enter_context(tc.tile_pool(name="wp", bufs=1))
        opool = ctx.enter_context(tc.tile_pool(name="op", bufs=16))
        ppool = ctx.enter_context(tc.tile_pool(name="pp", bufs=8, space="PSUM"))

        wt = wpool.tile([128, 6, 128], mybir.dt.float16)
        nc.sync.dma_start(wt[:], w.ap())

        xa = x.ap()
        ya = y.ap()

        # Input bands: each HWDGE ring is FIFO per engine and each DMA has
        # multi-us push->data->receipt latency, so the bands needed first
        # must be FIRST on their ring.  Image 0 alternates bands across the
        # sync and scalar rings (both rings work on it concurrently);
        # images 1-3 queue strictly behind.  Chunks wait only on the bands
        # they actually read (range-based hazard tracking).
        xtiles = [
            xpool.tile([128, XLEN], mybir.dt.float16, name="xt", tag="xt")
            for _ in range(B_CORE)
        ]

        def load(eng, b, lo, hi):
            end = hi * W if hi < H else XLEN
            eng.dma_start(xtiles[b][:, lo * W : end], xa[b][:, lo * W : end])

        # image 0 alone owns both rings + HBM read bandwidth at the start;
        # images 1-3 are emitted inside the group loop below, so their ring
        # entries sit behind sem-gated group work and only start once the
        # head-critical loads are done.
        load(nc.sync, 0, 0, 6)
        load(nc.scalar, 0, 6, 16)
        load(nc.sync, 0, 16, 30)
        load(nc.scalar, 0, 30, 50)
        load(nc.sync, 0, 50, 80)
        load(nc.scalar, 0, 80, 112)

        # Warm-up: the first real matmul can't start until the runtime
        # preamble + weight/band DMAs complete (~11us).  Run dependency-free
        # dummy matmuls on a zeroed scratch tile in that window so the PE HAM
        # clock-gate reaches 2.4 GHz (and pays its 1.2 GHz cold ramp) before
        # real work arrives.
        scratch = wpool.tile([128, 448], mybir.dt.float16)
        nc.vector.memset(scratch[:], 0.0)
        # shares the 8-bank "pt" rotation; its bank is reused by the 8th real
        # tile of group 0, whose WAR dep lands after the warm-up is long done
        pdummy = ppool.tile([128, 448], mybir.dt.float32, name="pt", tag="pt")
        N_WARM = 17
        for i in range(N_WARM):
            nc.tensor.matmul(
                pdummy[:],
                scratch[:, 0:128],
                scratch[:],
                start=(i == 0),
                stop=(i == N_WARM - 1),
                skip_group_check=True,
            )

        for g in range(n_groups):
            # stagger images 1-3 input loads: emitted after group g-1's
            # sem-gated instructions on each ring, so they queue behind the
            # head-critical image-0 bands instead of competing with them.
            # Images 2-3 go early (g1/g2): by then the head is past, and
            # late arrival of image 2 was the main mid-stream stall on
            # congestion-unlucky cores.
            # both halves ride the scalar ring: after the head it carries
            # no other DMAs, so these sems never queue behind the sem-gated
            # output pushes that congest the sync ring mid-stream (image 1
            # is emitted inside group 0 below, so its push lands in the
            # ring-idle window right after the head bands)
            if 2 <= g <= 3:
                load(nc.scalar, g, 0, 56)
                load(nc.scalar, g, 56, 112)
            gchunks = chunks[g * 8 : (g + 1) * 8]
            pts = [
                ppool.tile([128, 448], mybir.dt.float32, name="pt", tag="pt")
                for _ in range(8)
            ]
            def mm_ky01(j, kx):
                b, y0, rows = gchunks[j]
                n = rows * W
                nc.tensor.matmul(
                    pts[j][:, 0 : n - 2],
                    wt[:, kx, :],
                    xtiles[b][:, y0 * W + kx : y0 * W + kx + n - 2],
                    start=(kx == 0),
                    stop=False,
                    skip_group_check=True,
                )

            def mm_ky2(j, kx):
                # K=64: row-group 0-1 via copy A for even chunks, row-group
                # 2-3 via copy B for odd chunks; an even/odd pair runs
                # concurrently in one matmul slot
                b, y0, rows = gchunks[j]
                n = rows * W
                if j % 2 == 0:
                    lhsT = wt[0:64, 3 + kx, :]
                    rhs = xtiles[b][
                        0:64, (y0 + 2) * W + kx : (y0 + 2) * W + kx + n - 2
                    ]
                else:
                    lhsT = wt[64:128, 3 + kx, :]
                    rhs = xtiles[b][
                        64:128, (y0 + 1) * W + kx : (y0 + 1) * W + kx + n - 2
                    ]
                nc.tensor.matmul(
                    pts[j][:, 0 : n - 2],
                    lhsT,
                    rhs,
                    start=False,
                    stop=(kx == 2),
                    skip_group_check=True,
                )

            def emit_output(lo, hi, eng):
                # copy chunks [lo,hi) of the group into one staging tile and
                # DMA it out as a single contiguous write
                ochunks = gchunks[lo:hi]
                total_rows = sum(r for _, _, r in ochunks)
                ot = opool.tile([128, 16 * W], mybir.dt.float16, tag="ot")
                off = 0
                for jj, (b, y0, rows) in enumerate(ochunks):
                    j = lo + jj
                    n = rows * W
                    if j % 2 == 0:
                        nc.vector.tensor_copy(
                            ot[:, off : off + n - 2], pts[j][:, 0 : n - 2]
                        )
                    else:
                        nc.scalar.copy(ot[:, off : off + n - 2], pts[j][:, 0 : n - 2])
                    off += n
                b0, y00, _ = ochunks[0]
                assert all(b == b0 for b, _, _ in ochunks)
                assert ochunks[-1][1] + ochunks[-1][2] - y00 == total_rows
                eng.dma_start(
                    ya[b0].rearrange("c h w -> c (h w)")[
                        :, y00 * W : y00 * W + total_rows * W
                    ],
                    ot[:, 0 : total_rows * W],
                )

            if g == 0 or g == n_groups - 1:
                # First and last group run chunk-pair-major: slightly worse
                # LDWEIGHTS overlap (weights change every other matmul), but
                # in group 0 it paces input-row consumption to the DMA
                # delivery rate (no stall, no HAM re-throttle), and in the
                # last group it spreads the PSUM stops so the final output
                # drain overlaps the group's own compute.
                for p in range(4):
                    for kx in range(3):
                        mm_ky01(2 * p, kx)
                        mm_ky01(2 * p + 1, kx)
                    for kx in range(3):
                        mm_ky2(2 * p, kx)
                        mm_ky2(2 * p + 1, kx)
                    if g == n_groups - 1:
                        # final drain is latency-critical: emit each pair's
                        # output right after its stop, alternating rings, so
                        # the drain overlaps the group's own compute; the
                        # very last pair splits per-chunk so only the tiny
                        # 2-row chunk's write sits on the critical tail
                        if p == 3:
                            emit_output(6, 7, nc.scalar)
                            emit_output(7, 8, nc.sync)
                        else:
                            emit_output(
                                2 * p, 2 * p + 2, nc.sync if p % 2 else nc.scalar
                            )
                if g == 0:
                    load(nc.scalar, 1, 0, 56)
                    load(nc.scalar, 1, 56, 112)
                    emit_output(0, 4, nc.sync)
                    emit_output(4, 8, nc.sync)
            else:
                # steady state runs kx-major: 8 consecutive matmuls share
                # stationary weights, so every LDWEIGHTS hides under the
                # previous matmul's stream
                for kx in range(3):
                    for j in range(8):
                        mm_ky01(j, kx)
                # ky2 in two half-group sweeps: half 0's PSUM stops land
                # ~1.1us before group end, so its copies drain early and the
                # next group never waits on a bank (2 pair-slots per sub-
                # phase still hide the two 94ns weight loads)
                for h in range(2):
                    for kx in range(3):
                        for j in range(4 * h, 4 * h + 4):
                            mm_ky2(j, kx)
                # batch outputs per 4-chunk half: one contiguous DMA each
                # (small per-chunk DMAs are descriptor-dominated)
                emit_output(0, 4, nc.sync)
                emit_output(4, 8, nc.sync)

    nc.compile()
    return nc


def _get_nc():
    global _NC
    if _NC is None:
        _NC = _build()
    return _NC


def _prep_weights(weights: np.ndarray) -> np.ndarray:
    # w6[ci,      kx, co] = w[co, ci, ky=0, kx]   (fused ky=0/1 planes)
    # w6[64+ci,   kx, co] = w[co, ci, ky=1, kx]
    # w6[ci,    3+kx, co] = w[co, ci, ky=2, kx]   (ky=2, A-half row group)
    # w6[64+ci, 3+kx, co] = w[co, ci, ky=2, kx]   (ky=2, B-half row group)
    w = np.asarray(weights, dtype=np.float32)
    wt = w.transpose(1, 2, 3, 0)  # [ci, ky, kx, co]
    w6 = np.zeros((128, 6, 128), np.float32)
    w6[0:64, 0:3, :] = wt[:, 0, :, :]
    w6[64:128, 0:3, :] = wt[:, 1, :, :]
    w6[0:64, 3:6, :] = wt[:, 2, :, :]
    w6[64:128, 3:6, :] = wt[:, 2, :, :]
    return w6.astype(np.float16)


def kernel(input_image: np.ndarray, weights: np.ndarray, _trace: bool = False):
    from concourse.bass_utils import run_bass_kernel_spmd

    nc = _get_nc()
    x16 = np.asarray(input_image).astype(np.float16)  # [32, 64, 112, 112]
    xd = np.zeros((B_FULL, 128, XLEN), np.float16)
    xd[:, :C_IN, : H * W] = x16.reshape(B_FULL, C_IN, H * W)
    xd[:, C_IN:, : (H - 1) * W] = x16[:, :, 1:, :].reshape(B_FULL, C_IN, -1)
    w6 = _prep_weights(weights)
    in_maps = [
        {"x": xd[B_CORE * i : B_CORE * (i + 1)], "w": w6} for i in range(N_CORES)
    ]
    res = run_bass_kernel_spmd(
        nc, in_maps, core_ids=list(range(N_CORES)), trace=_trace
    )
    out = np.concatenate(
        [res.results[i]["y"][:, :, :, :OW] for i in range(N_CORES)], axis=0
    ).astype(np.float32)
    if _trace:
        return out, res
    return out
